# revision 1
# baseline (speedup 1.0000x reference)
"""FP8Linear kernel for Trainium2 (Bass/Tile), distributed over 8 NeuronCores.

Computation (matches the nn.Module reference):
    x:  [B=2, S=4096, K=4096] f32  -> x2d [M=8192, K]
    xq = tile_quant_dequant(x2d)    # per-row 1x64 chunks, fp8 e4m3fn round-trip
    wq = block_quant_dequant(w)     # 64x64 blocks of w [N=4096, K]
    out = f32(bf16(xq @ wq.T)) + bias  -> [B, S, N]

Distribution: 4x2 grid over (M, N). Each core independently computes a
[2048, 2048] output shard (data-parallel over rows, tensor-parallel over
out_features) -- no collectives.

Per-core pipeline:
  1. quantize x/w in natural layout into bf16 DRAM scratch:
       amax (1x64 chunk reduce on DVE; for w also a 64-partition-group max
       via PE-transpose + DVE reduce, rebroadcast through a tiny DRAM bounce)
       s2 = max(amax,1e-12)/224
       q  = fp8_e4m3_trn(x / s2)     (half of the e4m3fn grid: TRN fp8e4
                                      max-normal is 240, not 448; x/(2s)
                                      with dequant by 2s reproduces the
                                      e4m3fn RNE grid exactly for normals)
       dq = bf16(q * s2)             (on GpSimd)
  2. dq is re-loaded K-major with the DMA-xbar transpose: x into [128,
     K/128, 512] cache units (3 bufs), w per n-panel into [128, K/128, 512].
  3. bf16 matmuls with f32 PSUM accumulation (K on partitions), ACT
     evacuates PSUM with a bf16 cast, GpSimd adds the (f32) bias, DMA out.
"""

import os
import sys

sys.path.insert(0, "/opt/trn_rl_repo")

import numpy as np
from contextlib import ExitStack

import concourse.bass as bass
import concourse.mybir as mybir
import concourse.tile as tile
from concourse import bacc
from concourse.bass import ts
from concourse.masks import make_identity

P = 128
QT = 64  # quantization tile (1x64 for x, 64x64 for w)

# full-problem dims
B, S, K, N = 2, 4096, 4096, 4096
M = B * S
# sharding grid
GRID_M, GRID_N = 4, 2
M_SH, N_SH = M // GRID_M, N // GRID_N  # 2048, 2048

F32 = mybir.dt.float32
BF16 = mybir.dt.bfloat16
FP8 = mybir.dt.float8e4

NQ = 4  # natural tiles are processed in K/NQ-wide quarters


class Ctx:
    """Bag of pools / constants shared by the emit helpers."""


def _qdq_quarter(cx, nc, nat, dq, s2, rs2, kb0, kb, dq_engine=None):
    """fp8 round-trip of one loaded quarter: dq = bf16(fp8(nat * rs2) * s2),
    using scale columns [kb0, kb0+kb) of s2/rs2.  The dequant multiply goes
    on dq_engine (gpsimd for w, vector for x, so the two quant streams pace
    different engines)."""
    nat_v = nat[:].rearrange("p (c t) -> p c t", t=QT)
    q = cx.q.tile([P, kb * QT], FP8, tag="q")
    q_v = q[:].rearrange("p (c t) -> p c t", t=QT)
    dq_v = dq[:].rearrange("p (c t) -> p c t", t=QT)
    nc.vector.tensor_tensor(
        q_v, nat_v,
        rs2[:, kb0 : kb0 + kb, None].to_broadcast((P, kb, QT)),
        op=mybir.AluOpType.mult,
    )
    (dq_engine or nc.gpsimd).tensor_tensor(
        dq_v, q_v,
        s2[:, kb0 : kb0 + kb, None].to_broadcast((P, kb, QT)),
        op=mybir.AluOpType.mult,
    )


def _emit_x_row(cx, nc, x, xdq, mi, row0, kdim):
    """Quantize x rows [row0, row0+128) into xdq rows [mi*128, +128),
    one [128, kdim/NQ] quarter at a time."""
    kh = kdim // NQ
    kb = kh // QT
    if True:
        for qq in range(NQ):
            nat = cx.nat.tile([P, kh], F32, tag="nat")
            nc.sync.dma_start(
                nat[:], x[row0 : row0 + P, qq * kh : (qq + 1) * kh]
            )
            a = cx.amax.tile([P, kb], F32, tag="amax_x")
            nc.vector.tensor_reduce(
                a[:], nat[:].rearrange("p (c t) -> p c t", t=QT),
                axis=mybir.AxisListType.X, op=mybir.AluOpType.max,
                apply_absolute_value=True,
            )
            s2 = cx.scale.tile([P, kb], F32, tag="s2x")
            rs2 = cx.scale.tile([P, kb], F32, tag="rs2x")
            nc.vector.tensor_scalar(
                s2[:], a[:], 1e-12, 1.0 / 224.0,
                op0=mybir.AluOpType.max, op1=mybir.AluOpType.mult,
            )
            nc.vector.reciprocal(rs2[:], s2[:])
            dq = cx.dq.tile([P, kh], BF16, tag="dq")
            _qdq_quarter(cx, nc, nat, dq, s2, rs2, 0, kb)
            nc.scalar.dma_start(
                xdq[mi * P : mi * P + P, qq * kh : (qq + 1) * kh], dq[:]
            )


def _emit_w_row(cx, nc, w, wdq, ni, row0, kdim):
    """Quantize one w row-tile [row0, row0+128) x [0, kdim) with 64x64 block
    scales into wdq rows [ni*128, +128).  Loads NQ quarters, computes all
    block scales with one PE transpose + one DRAM bounce pair, then
    fp8-round-trips each quarter."""
    kh = kdim // NQ
    kbq = kh // QT
    kbf = kdim // QT
    nats = []
    a = cx.amax.tile([P, kbf], F32, tag="amax_w")
    for qq in range(NQ):
        nat = cx.natw.tile([P, kh], F32, tag="natw")
        nats.append(nat)
        nc.sync.dma_start(
            nat[:], w[row0 : row0 + P, qq * kh : (qq + 1) * kh]
        )
        nc.vector.tensor_reduce(
            a[:, qq * kbq : (qq + 1) * kbq],
            nat[:].rearrange("p (c t) -> p c t", t=QT),
            axis=mybir.AxisListType.X, op=mybir.AluOpType.max,
            apply_absolute_value=True,
        )
    # 64-partition-group max via PE transpose (f32) + DVE reduce; the
    # [kbf, 2] block scales go through a tiny DRAM bounce to become
    # partition-broadcast rows.
    at_ps = cx.tpsum.tile([kbf, P], F32, tag="at_ps")
    nc.tensor.transpose(at_ps[:], a[:], cx.ident_f32[:])
    r = cx.amax.tile([kbf, 2], F32, tag="r_blk")
    nc.vector.tensor_reduce(
        r[:], at_ps[:].rearrange("p (g t) -> p g t", t=QT),
        axis=mybir.AxisListType.X, op=mybir.AluOpType.max,
    )
    s2blk = cx.scale.tile([kbf, 2], F32, tag="s2blk")
    nc.vector.tensor_scalar(
        s2blk[:], r[:], 1e-12, 1.0 / 224.0,
        op0=mybir.AluOpType.max, op1=mybir.AluOpType.mult,
    )
    s2 = cx.scale.tile([P, kbf], F32, tag="s2w")
    rs2 = cx.scale.tile([P, kbf], F32, tag="rs2w")
    for nb in (0, 1):
        s2row_dram = cx.dram_small.tile([1, kbf], F32, tag="s2row_dram")
        nc.gpsimd.dma_start(s2row_dram[:], s2blk[:, nb : nb + 1])
        nc.gpsimd.dma_start(
            s2[nb * QT : (nb + 1) * QT, :],
            s2row_dram[:].to_broadcast((QT, kbf)),
        )
    nc.vector.reciprocal(rs2[:], s2[:])
    for qq in range(NQ):
        dq = cx.dq.tile([P, kh], BF16, tag="dq")
        _qdq_quarter(cx, nc, nats[qq], dq, s2, rs2, qq * kbq, kbq)
        nc.scalar.dma_start(
            wdq[ni * P : ni * P + P, qq * kh : (qq + 1) * kh], dq[:]
        )


def _emit_xT_unit(cx, nc, xdq, u0, usz, kdim):
    """xbar transpose-load xdq rows [u0, u0+usz) into a K-major unit
    [128, kdim/128, usz]."""
    xT = cx.xT.tile([P, kdim // P, usz], BF16, tag="xT")
    for kb in range(kdim // P):
        nc.scalar.dma_start(
            xT[:, kb, :], xdq[u0 : u0 + usz, ts(kb, P)], transpose=True
        )
    return xT


def _emit_wT(cx, nc, wdq, n_panel, kdim):
    kbt = kdim // P
    wT = cx.wT.tile([P, kbt, n_panel], BF16, tag="wT")
    for kb in range(kbt):
        nc.scalar.dma_start(wT[:, kb, :], wdq[:, ts(kb, P)], transpose=True)
    return wT


def _emit_sweep(cx, nc, out, xTs, usz, wT, mo, pn, mo_chunk,
                n_panel, kdim):
    """One (mo, panel) sweep: matmuls for all m-tiles of this mo chunk,
    evacuate + bias + store."""
    kbt = kdim // P
    bias_bc = cx.biasp.tile([P, n_panel], F32, tag="biasp")
    nc.sync.dma_start(
        bias_bc[:], cx.b[:, ts(pn, n_panel)].to_broadcast((P, n_panel))
    )
    for mi in range(mo_chunk // P):
        xT = xTs[mi // (usz // P)]
        msub = mi % (usz // P)
        ps = cx.mpsum.tile([P, n_panel], F32, tag="mpsum")
        for kb in range(kbt):
            nc.tensor.matmul(
                ps[:], xT[:, kb, ts(msub, P)], wT[:, kb, :],
                start=(kb == 0), stop=(kb == kbt - 1),
            )
        ob = cx.evac.tile([P, n_panel], BF16, tag="evac")
        nc.scalar.copy(ob[:], ps[:])
        of = cx.outf.tile([P, n_panel], F32, tag="outf")
        nc.gpsimd.tensor_tensor(
            of[:], ob[:], bias_bc[:], op=mybir.AluOpType.add
        )
        row = mo * mo_chunk + mi * P
        nc.sync.dma_start(out[row : row + P, ts(pn, n_panel)], of[:])


def fp8_linear_core_kernel(tc, out, x, w, b, m_sh, n_sh, kdim,
                           mo_chunk=1024, n_panel=512):
    """Per-core kernel: out [m_sh, n_sh] f32 = bf16(xq @ wq.T) + b.
    x [m_sh, kdim] f32, w [n_sh, kdim] f32, b [1, n_sh] f32."""
    nc = tc.nc
    ctx = tc.ctx  # ExitStack owned by the caller
    assert m_sh % mo_chunk == 0 and n_sh % n_panel == 0 and kdim % P == 0

    cx = Ctx()
    cx.n_panels = n_sh // n_panel
    n_mo = m_sh // mo_chunk
    usz = min(512, mo_chunk)  # xT unit width (m rows per cache tile)
    units = mo_chunk // usz

    cx.nat = ctx.enter_context(tc.tile_pool(name="nat", bufs=5))
    cx.natw = ctx.enter_context(tc.tile_pool(name="natw", bufs=7))
    cx.q = ctx.enter_context(tc.tile_pool(name="q", bufs=6))
    cx.dq = ctx.enter_context(tc.tile_pool(name="dq", bufs=5))
    cx.amax = ctx.enter_context(tc.tile_pool(name="amax", bufs=6))
    cx.scale = ctx.enter_context(tc.tile_pool(name="scale", bufs=5))
    cx.xT = ctx.enter_context(tc.tile_pool(name="xT", bufs=units))
    cx.wT = ctx.enter_context(tc.tile_pool(name="wT", bufs=2))
    cx.mpsum = ctx.enter_context(tc.tile_pool(name="mpsum", bufs=6, space="PSUM"))
    cx.tpsum = ctx.enter_context(tc.tile_pool(name="tpsum", bufs=2, space="PSUM"))
    cx.evac = ctx.enter_context(tc.tile_pool(name="evac", bufs=2))
    cx.outf = ctx.enter_context(tc.tile_pool(name="outf", bufs=2))
    cx.const = ctx.enter_context(tc.tile_pool(name="const", bufs=1))
    cx.dram = ctx.enter_context(
        tc.tile_pool(name="scratch", bufs=cx.n_panels, space="DRAM")
    )
    cx.dram_small = ctx.enter_context(
        tc.tile_pool(name="scratch_s", bufs=8, space="DRAM")
    )

    cx.ident_f32 = cx.const.tile([P, P], F32)
    make_identity(nc, cx.ident_f32)

    cx.biasp = ctx.enter_context(tc.tile_pool(name="biasp", bufs=2))
    cx.b = b

    # Producer/consumer split.  Production round-robins one w row-tile
    # (Pool-paced: gpsimd dequant + SWDGE scale bounce) with one x row-tile
    # (DVE-paced dequant), so the two quant streams saturate different
    # engines concurrently; the matmul sweeps then just consume.  xdq
    # scratch is one DRAM tile per xT unit so the transpose-loads only
    # depend on their own producers.
    n_units_total = m_sh // usz
    rows_per_unit = usz // P
    wdq = [None] * cx.n_panels
    xdq = [None] * n_units_total
    n_wrows = n_sh // P
    n_xrows = m_sh // P

    def w_row(i):
        pn = i // (n_panel // P)
        if wdq[pn] is None:
            wdq[pn] = cx.dram.tile([n_panel, kdim], BF16, tag="wdq",
                                   bufs=cx.n_panels, name=f"wdq{pn}")
        _emit_w_row(cx, nc, w, wdq[pn], i % (n_panel // P), i * P, kdim)

    def x_unit(u):
        xdq[u] = cx.dram.tile([usz, kdim], BF16, tag="xdq",
                              bufs=n_units_total, name=f"xdq{u}")
        for r in range(rows_per_unit):
            _emit_x_row(cx, nc, x, xdq[u], r, u * usz + r * P, kdim)

    # Production in priority order (panel 0, first chunk's x units, panel 1,
    # remaining x units, remaining panels), with each consumer transpose-load
    # emitted immediately after its producer so the scalar HWDGE ring's FIFO
    # order matches dependency order (no head-of-line blocking).
    wr = n_panel // P
    wTs = [None] * cx.n_panels
    xTs = [None] * n_units_total

    def w_panel(pn, transpose):
        for i in range(pn * wr, (pn + 1) * wr):
            w_row(i)
        if transpose:
            wTs[pn] = _emit_wT(cx, nc, wdq[pn], n_panel, kdim)

    w_panel(0, True)
    for u in range(units):
        x_unit(u)
        xTs[u] = _emit_xT_unit(cx, nc, xdq[u], 0, usz, kdim)
    w_panel(1, cx.n_panels > 2)  # skip transpose if it would need slot 0
    for u in range(units, n_units_total):
        x_unit(u)
    for pn in range(2, cx.n_panels):
        w_panel(pn, False)

    for mo in range(n_mo):
        if mo > 0:
            for u in range(units):
                xTs[mo * units + u] = _emit_xT_unit(
                    cx, nc, xdq[mo * units + u], 0, usz, kdim)
        for pn in range(cx.n_panels):
            # keep one wT panel transposing ahead of the sweeps
            nxt = pn + 1
            if wTs[pn] is None:
                wTs[pn] = _emit_wT(cx, nc, wdq[pn], n_panel, kdim)
            wT = wTs[pn]
            wTs[pn] = None
            if nxt < cx.n_panels and wTs[nxt] is None:
                wTs[nxt] = _emit_wT(cx, nc, wdq[nxt], n_panel, kdim)
            _emit_sweep(cx, nc, out, xTs[mo * units : (mo + 1) * units],
                        usz, wT, mo, pn, mo_chunk, n_panel, kdim)


def build_core_bass(m_sh=M_SH, n_sh=N_SH, kdim=K, mo_chunk=1024, n_panel=512,
                    num_devices=8):
    """Build the (SPMD-identical) per-core Bass program."""
    nc = bacc.Bacc(
        "TRN2", target_bir_lowering=False, debug=False, num_devices=num_devices
    )
    x = nc.dram_tensor("x", [m_sh, kdim], F32, kind="ExternalInput").ap()
    w = nc.dram_tensor("w", [n_sh, kdim], F32, kind="ExternalInput").ap()
    b = nc.dram_tensor("b", [1, n_sh], F32, kind="ExternalInput").ap()
    out = nc.dram_tensor("out", [m_sh, n_sh], F32, kind="ExternalOutput").ap()
    with tile.TileContext(nc) as tc:
        with ExitStack() as stack:
            tc.ctx = stack
            fp8_linear_core_kernel(tc, out, x, w, b, m_sh, n_sh, kdim,
                                   mo_chunk=mo_chunk, n_panel=n_panel)
    nc.compile()
    return nc


_NC_CACHE = []


def _get_nc():
    if not _NC_CACHE:
        _NC_CACHE.append(build_core_bass())
    return _NC_CACHE[0]


def kernel(x, weight, bias):
    """Full-problem entry point: x [2,4096,4096] f32, weight [4096,4096] f32,
    bias [4096] f32 -> [2,4096,4096] f32."""
    from concourse.bass_utils import run_bass_kernel_spmd

    x2d = np.ascontiguousarray(x.reshape(M, K), dtype=np.float32)
    weight = np.ascontiguousarray(weight, dtype=np.float32)
    bias2d = np.ascontiguousarray(bias.reshape(1, N), dtype=np.float32)

    nc = _get_nc()

    in_maps = []
    for core in range(8):
        mi, nj = core // GRID_N, core % GRID_N
        in_maps.append(
            {
                "x": np.ascontiguousarray(x2d[mi * M_SH : (mi + 1) * M_SH]),
                "w": np.ascontiguousarray(weight[nj * N_SH : (nj + 1) * N_SH]),
                "b": np.ascontiguousarray(bias2d[:, nj * N_SH : (nj + 1) * N_SH]),
            }
        )

    res = run_bass_kernel_spmd(nc, in_maps, core_ids=list(range(8)))
    global LAST_EXEC_TIME_NS
    LAST_EXEC_TIME_NS = res.exec_time_ns

    out = np.empty((M, N), dtype=np.float32)
    for core in range(8):
        mi, nj = core // GRID_N, core % GRID_N
        out[mi * M_SH : (mi + 1) * M_SH, nj * N_SH : (nj + 1) * N_SH] = (
            res.results[core]["out"]
        )
    return out.reshape(B, S, N)



# revision 3
# speedup vs baseline: 1.4624x; 1.4624x over previous
"""FP8Linear kernel for Trainium2 (Bass/Tile), distributed over 8 NeuronCores.

Computation (matches the nn.Module reference):
    x:  [B=2, S=4096, K=4096] f32  -> x2d [M=8192, K]
    xq = tile_quant_dequant(x2d)    # per-row 1x64 chunks, fp8 e4m3fn round-trip
    wq = block_quant_dequant(w)     # 64x64 blocks of w [N=4096, K]
    out = f32(bf16(xq @ wq.T)) + bias  -> [B, S, N]

Distribution: 4x2 grid over (M, N). Each core independently computes a
[2048, 2048] output shard (data-parallel over rows, tensor-parallel over
out_features) -- no collectives.

v2 design (fully on-chip, PE-transpose based):
  * natural-layout quantization: per 128-row tile, amax per 1x64 chunk
    (DVE reduce); for w, 64x64 block scales via PE-transpose of the
    chunk-amax + DVE reduce + a tiny DRAM broadcast bounce.
    s2 = max(amax,1e-12)/224; q = fp8e4(v*rs2); dq = bf16(q*s2).
    (TRN fp8e4 max-normal is 240: v/(2s) with dequant by 2s reproduces
    the e4m3fn RNE grid for normals.)
  * dq tiles are transposed on-chip by the tensor engine (128x128
    transposes, bf16 through PSUM, ACT evacuates groups of 4) -- no
    DRAM round-trip, no DMA-xbar transposes.
  * xT for all 16 m-tiles stays resident in SBUF ([128,32,128] bf16
    each); wT panels [128,32,512] are double-buffered and prepped one
    panel ahead.
  * main loop is panel-outer / m-tile-inner: 512 back-to-back matmuls
    per panel keep the PE warm (HAM at 2.4 GHz); PSUM f32 accumulation
    over 32 k-tiles; ACT evacuates f32 (skipping the reference's bf16
    round-trip of the output adds ~2e-3 relative error, well inside
    tolerance); bias is all-zeros in this problem (checked at run time;
    a general variant with a DVE bias add is built on demand).
"""

import sys

sys.path.insert(0, "/opt/trn_rl_repo")

import numpy as np
from contextlib import ExitStack

import concourse.bass as bass
import concourse.mybir as mybir
import concourse.tile as tile
from concourse import bacc
from concourse.bass import ts
from concourse.masks import make_identity

P = 128
QT = 64  # quantization chunk (1x64 for x, 64x64 blocks for w)

# full-problem dims
B, S, K, N = 2, 4096, 4096, 4096
M = B * S
# sharding grid
GRID_M, GRID_N = 4, 2
M_SH, N_SH = M // GRID_M, N // GRID_N  # 2048, 2048

F32 = mybir.dt.float32
BF16 = mybir.dt.bfloat16
FP8 = mybir.dt.float8e4

KQ = 1024          # processing quarter width (along K)
NQ = K // KQ       # 4 quarters per row-tile
CB = KQ // QT      # 16 scale chunks per quarter
KBF = K // QT      # 64 scale chunks per full row
KT = K // P        # 32 k-tiles of 128
N_PANEL = 256
N_PANELS = N_SH // N_PANEL   # 4
N_MT = M_SH // P             # 16 m-tiles
W_RT = N_PANEL // P          # 4 w row-tiles per panel


class Ctx:
    """Bag of pools / constants shared by the emit helpers."""


def _quant_quarter(cx, nc, nat, dq, s2, rs2, c0, alt):
    """fp8 round-trip of one loaded quarter into dq (bf16), using scale
    columns [c0, c0+CB) of s2/rs2.  alt flips which of DVE/GpSimd does the
    quant vs dequant multiply so the two streams stay balanced."""
    nat_v = nat[:].rearrange("p (c t) -> p c t", t=QT)
    q = cx.q.tile([P, KQ], FP8, tag="q")
    q_v = q[:].rearrange("p (c t) -> p c t", t=QT)
    dq_v = dq[:].rearrange("p (c t) -> p c t", t=QT)
    e0, e1 = (nc.vector, nc.gpsimd) if alt else (nc.gpsimd, nc.vector)
    e0.tensor_tensor(
        q_v, nat_v,
        rs2[:, c0 : c0 + CB, None].to_broadcast((P, CB, QT)),
        op=mybir.AluOpType.mult,
    )
    e1.tensor_tensor(
        dq_v, q_v,
        s2[:, c0 : c0 + CB, None].to_broadcast((P, CB, QT)),
        op=mybir.AluOpType.mult,
    )


def _transpose_quarter(cx, nc, dq, dst, col0, qq):
    """PE-transpose one dq quarter [128, KQ] into dst[:, kt0:kt0+8,
    col0:col0+128] (a [128, 32, ncol] K-major cache tile)."""
    kt0 = qq * (KQ // P)
    for g in range(KQ // P // 4):  # groups of 4 k-subtiles per PSUM tile
        tp = cx.tp.tile([P, 4 * P], BF16, tag="tp")
        for i in range(4):
            nc.tensor.transpose(
                tp[:, i * P : (i + 1) * P],
                dq[:, (g * 4 + i) * P : (g * 4 + i + 1) * P],
                cx.ident_bf16[:],
            )
        nc.scalar.copy(
            dst[:, kt0 + g * 4 : kt0 + g * 4 + 4, col0 : col0 + P],
            tp[:].rearrange("p (i m) -> p i m", m=P),
        )


def _emit_x_tile(cx, nc, x, mt):
    """Quantize + transpose x rows [mt*128, +128) into resident xT[mt]."""
    xT = cx.xT.tile([P, KT, P], BF16, tag="xT", name=f"xT{mt}")
    cx.xTs[mt] = xT
    row0 = mt * P
    for qq in range(NQ):
        nat = cx.nat.tile([P, KQ], F32, tag="nat")
        nc.sync.dma_start(nat[:], x[row0 : row0 + P, qq * KQ : (qq + 1) * KQ])
        a = cx.amax.tile([P, CB], F32, tag="amax_x")
        nc.vector.tensor_reduce(
            a[:], nat[:].rearrange("p (c t) -> p c t", t=QT),
            axis=mybir.AxisListType.X, op=mybir.AluOpType.max,
            apply_absolute_value=True,
        )
        s2 = cx.scale.tile([P, CB], F32, tag="s2x")
        rs2 = cx.scale.tile([P, CB], F32, tag="rs2x")
        nc.vector.tensor_scalar(
            s2[:], a[:], 1e-12, 1.0 / 224.0,
            op0=mybir.AluOpType.max, op1=mybir.AluOpType.mult,
        )
        nc.vector.reciprocal(rs2[:], s2[:])
        dq = cx.dq.tile([P, KQ], BF16, tag="dq")
        _quant_quarter(cx, nc, nat, dq, s2, rs2, 0, alt=(qq % 2 == 0))
        _transpose_quarter(cx, nc, dq, xT, 0, qq)


def _emit_w_tile(cx, nc, w, wT, rt_global, col0):
    """Quantize + transpose w rows [rt_global*128, +128) into
    wT[:, :, col0:col0+128] with 64x64 block scales."""
    row0 = rt_global * P
    nats = []
    a = cx.amax.tile([P, KBF], F32, tag="amax_w")
    for qq in range(NQ):
        nat = cx.nat.tile([P, KQ], F32, tag="nat")
        nats.append(nat)
        nc.sync.dma_start(nat[:], w[row0 : row0 + P, qq * KQ : (qq + 1) * KQ])
        nc.vector.tensor_reduce(
            a[:, qq * CB : (qq + 1) * CB],
            nat[:].rearrange("p (c t) -> p c t", t=QT),
            axis=mybir.AxisListType.X, op=mybir.AluOpType.max,
            apply_absolute_value=True,
        )
    # 64-partition-group max via PE transpose (f32) + DVE reduce; the
    # [KBF, 2] block scales bounce through DRAM to become
    # partition-broadcast rows.
    at_ps = cx.tps.tile([KBF, P], F32, tag="at_ps")
    nc.tensor.transpose(at_ps[:], a[:], cx.ident_f32[:])
    r = cx.amax.tile([KBF, 2], F32, tag="r_blk")
    nc.vector.tensor_reduce(
        r[:], at_ps[:].rearrange("p (g t) -> p g t", t=QT),
        axis=mybir.AxisListType.X, op=mybir.AluOpType.max,
    )
    s2blk = cx.scale.tile([KBF, 2], F32, tag="s2blk")
    nc.vector.tensor_scalar(
        s2blk[:], r[:], 1e-12, 1.0 / 224.0,
        op0=mybir.AluOpType.max, op1=mybir.AluOpType.mult,
    )
    s2 = cx.scale.tile([P, KBF], F32, tag="s2w")
    rs2 = cx.scale.tile([P, KBF], F32, tag="rs2w")
    for nb in (0, 1):
        s2row_dram = cx.dram_small.tile([1, KBF], F32, tag="s2row_dram")
        nc.gpsimd.dma_start(s2row_dram[:], s2blk[:, nb : nb + 1])
        nc.gpsimd.dma_start(
            s2[nb * QT : (nb + 1) * QT, :],
            s2row_dram[:].to_broadcast((QT, KBF)),
        )
    nc.vector.reciprocal(rs2[:], s2[:])
    for qq in range(NQ):
        dq = cx.dq.tile([P, KQ], BF16, tag="dq")
        _quant_quarter(cx, nc, nats[qq], dq, s2, rs2, qq * CB,
                       alt=(qq % 2 == 0))
        _transpose_quarter(cx, nc, dq, wT, col0, qq)


def _emit_w_panel(cx, nc, w, pn):
    wT = cx.wT.tile([P, KT, N_PANEL], BF16, tag="wT", name=f"wT{pn}")
    for rt in range(W_RT):
        _emit_w_tile(cx, nc, w, wT, pn * W_RT + rt, rt * P)
    return wT


def _emit_mm(cx, nc, out, b, wT, pn, mt):
    """One (panel, m-tile) accumulation sweep + evacuate + store."""
    ps = cx.mm.tile([P, N_PANEL], F32, tag="mm")
    xT = cx.xTs[mt]
    for kt in range(KT):
        nc.tensor.matmul(
            ps[:], xT[:, kt, :], wT[:, kt, :],
            start=(kt == 0), stop=(kt == KT - 1),
        )
    outf = cx.outf.tile([P, N_PANEL], F32, tag="outf")
    if cx.with_bias:
        nc.vector.tensor_tensor(
            outf[:], ps[:], cx.bias_bc[:, ts(pn, N_PANEL)],
            op=mybir.AluOpType.add,
        )
    else:
        nc.scalar.copy(outf[:], ps[:])
    nc.scalar.dma_start(out[mt * P : (mt + 1) * P, ts(pn, N_PANEL)], outf[:])


def fp8_linear_core_kernel(tc, out, x, w, b, with_bias):
    nc = tc.nc
    ctx = tc.ctx  # ExitStack owned by the caller

    cx = Ctx()
    cx.with_bias = with_bias
    cx.xTs = [None] * N_MT

    cx.nat = ctx.enter_context(tc.tile_pool(name="nat", bufs=4))
    cx.q = ctx.enter_context(tc.tile_pool(name="q", bufs=3))
    cx.dq = ctx.enter_context(tc.tile_pool(name="dq", bufs=4))
    cx.amax = ctx.enter_context(tc.tile_pool(name="amax", bufs=6))
    cx.scale = ctx.enter_context(tc.tile_pool(name="scale", bufs=8))
    cx.xT = ctx.enter_context(tc.tile_pool(name="xT", bufs=N_MT))
    cx.wT = ctx.enter_context(tc.tile_pool(name="wT", bufs=2))
    cx.outf = ctx.enter_context(tc.tile_pool(name="outf", bufs=3))
    cx.const = ctx.enter_context(tc.tile_pool(name="const", bufs=1))
    cx.mm = ctx.enter_context(tc.tile_pool(name="mm", bufs=5, space="PSUM"))
    cx.tp = ctx.enter_context(tc.tile_pool(name="tp", bufs=2, space="PSUM"))
    cx.tps = ctx.enter_context(tc.tile_pool(name="tps", bufs=1, space="PSUM"))
    cx.dram_small = ctx.enter_context(
        tc.tile_pool(name="scratch_s", bufs=8, space="DRAM")
    )

    cx.ident_f32 = cx.const.tile([P, P], F32)
    make_identity(nc, cx.ident_f32)
    cx.ident_bf16 = cx.const.tile([P, P], BF16)
    make_identity(nc, cx.ident_bf16)

    if with_bias:
        cx.biasp = ctx.enter_context(tc.tile_pool(name="biasp", bufs=1))
        cx.bias_bc = cx.biasp.tile([P, N_SH], F32)
        nc.sync.dma_start(cx.bias_bc[:], b[:].to_broadcast((P, N_SH)))

    # Prologue: first w panel, first three x tiles.
    wTs = [None, None]
    wTs[0] = _emit_w_panel(cx, nc, w, 0)
    for mt in range(3):
        _emit_x_tile(cx, nc, x, mt)

    # Main: panel-outer, m-tile-inner.  During panel 0 the remaining x
    # tiles stream in three m-tiles ahead of their matmuls; the next w
    # panel is prepped late in panel 0 (x quant paces the start) and
    # early in panels 1+.
    for pn in range(N_PANELS):
        for mt in range(N_MT):
            if pn == 0 and mt + 3 < N_MT:
                _emit_x_tile(cx, nc, x, mt + 3)
            if pn + 1 < N_PANELS and mt == (12 if pn == 0 else 0):
                wTs[(pn + 1) % 2] = _emit_w_panel(cx, nc, w, pn + 1)
            _emit_mm(cx, nc, out, b, wTs[pn % 2], pn, mt)


def build_core_bass(with_bias=False, num_devices=8):
    """Build the (SPMD-identical) per-core Bass program."""
    nc = bacc.Bacc(
        "TRN2", target_bir_lowering=False, debug=False, num_devices=num_devices
    )
    x = nc.dram_tensor("x", [M_SH, K], F32, kind="ExternalInput").ap()
    w = nc.dram_tensor("w", [N_SH, K], F32, kind="ExternalInput").ap()
    b = nc.dram_tensor("b", [1, N_SH], F32, kind="ExternalInput").ap()
    out = nc.dram_tensor("out", [M_SH, N_SH], F32, kind="ExternalOutput").ap()
    with tile.TileContext(nc) as tc:
        with ExitStack() as stack:
            tc.ctx = stack
            fp8_linear_core_kernel(tc, out, x, w, b, with_bias)
    nc.compile()
    return nc


_NC_CACHE = {}


def _get_nc(with_bias):
    if with_bias not in _NC_CACHE:
        _NC_CACHE[with_bias] = build_core_bass(with_bias=with_bias)
    return _NC_CACHE[with_bias]


def kernel(x, weight, bias):
    """Full-problem entry point: x [2,4096,4096] f32, weight [4096,4096] f32,
    bias [4096] f32 -> [2,4096,4096] f32."""
    from concourse.bass_utils import run_bass_kernel_spmd

    x2d = np.ascontiguousarray(x.reshape(M, K), dtype=np.float32)
    weight = np.ascontiguousarray(weight, dtype=np.float32)
    bias2d = np.ascontiguousarray(bias.reshape(1, N), dtype=np.float32)

    with_bias = bool(np.any(bias2d))
    nc = _get_nc(with_bias)

    in_maps = []
    for core in range(8):
        mi, nj = core // GRID_N, core % GRID_N
        in_maps.append(
            {
                "x": np.ascontiguousarray(x2d[mi * M_SH : (mi + 1) * M_SH]),
                "w": np.ascontiguousarray(weight[nj * N_SH : (nj + 1) * N_SH]),
                "b": np.ascontiguousarray(bias2d[:, nj * N_SH : (nj + 1) * N_SH]),
            }
        )

    res = run_bass_kernel_spmd(nc, in_maps, core_ids=list(range(8)))
    global LAST_EXEC_TIME_NS
    LAST_EXEC_TIME_NS = res.exec_time_ns

    out = np.empty((M, N), dtype=np.float32)
    for core in range(8):
        mi, nj = core // GRID_N, core % GRID_N
        out[mi * M_SH : (mi + 1) * M_SH, nj * N_SH : (nj + 1) * N_SH] = (
            res.results[core]["out"]
        )
    return out.reshape(B, S, N)
